# revision 20
# baseline (speedup 1.0000x reference)
"""Two-layer GAT on Trainium2 (8 NeuronCores, SPMD).

Strategy (graph/data parallel, dst-sharded):
- Nodes are sharded across 8 cores by contiguous destination ranges (6250 each).
- Phase 1 (replicated on every core): one fused matmul
  x @ [W1 | W1@att_src.T | W1@att_dst.T] produces per-node h, a_src, a_dst.
  Feature-table rows [h(256)|a_src(4)|pad] fp16 (768B) are written to two DRAM
  tables (lo: nodes < 32767, hi: rest) because the fast gather
  (InstDMAGatherAnt) takes int16 row indices.
- Phase 2: per core, edges (incl. self loops) grouped by dst, two passes by
  src range. In each pass the shard's dsts are sorted by pass-degree and
  packed into blocks of 128 (partition dim) x J[b] slots; padding points at a
  dummy table row whose a_src=-30000 so exp()==0. One dma_gather per block
  fetches all source rows; e=exp(lrelu(a_src+a_dst)+kneg) is expanded on the
  Scalar engine (which also emits the softmax denominator via accum_out, and
  kneg is a per-dst shift keeping exp() in fp16 range); messages are weighted
  on the Vector engine and pairwise-tree-summed over slots. Per-dst num|den
  partials go to DRAM (the H pass scatters into L-pass row order).
- Phase 3: combine passes, normalize, +bias, ELU, h2 = elu @ W2ext
  (transpose via TensorE). Per-node layer-1 results return to the host, which
  assembles the full layer-2 table (fp16, 256B rows) for launch 2.
- Launch 2 repeats phases 2/3 for the output layer (1 head, 40 classes).
"""
import sys

import numpy as np

sys.path.insert(0, "/opt/trn_rl_repo")

import concourse.bacc as bacc
import concourse.bass as bass
import concourse.mybir as mybir
from concourse import library_config
from concourse.bass import IndirectOffsetOnAxis
from concourse.bass_utils import run_bass_kernel_spmd
from concourse.masks import make_identity
from concourse.tile import TileContext

FP16 = mybir.dt.float16
F32 = mybir.dt.float32
I16 = mybir.dt.int16
I32 = mybir.dt.int32
AF = mybir.ActivationFunctionType
ALU = mybir.AluOpType

N = 50000
F_IN = 256
H = 4
C = 64
HC = H * C            # 256
NCLS = 40
SLOPE = 0.2
SH = 8
NS = N // SH          # 6250
NPAD = 50176          # 392 * 128
SPLIT = 32767         # nodes < SPLIT -> T_lo at row node+1 (row 0 = dummy)
LO_ROWS = 32768
HI_ROWS = NPAD - SPLIT + 1   # 17410 (last row = dummy)
HI_DUMMY = NPAD - SPLIT      # 17409
ROW1 = 384            # fp16 elems -> 768B
ROW2 = 128            # fp16 elems -> 256B
NBLK = (NS + 127) // 128     # 49
ASRC_DUMMY = -30000.0
SLAB = 28             # node blocks per phase-1 slab (392 = 14*28)
NSLAB = NPAD // (SLAB * 128)
RW = HC + H           # 260: table-row payload elems (layer 1)


# --------------------------------------------------------------------------
# host-side edge plan
# --------------------------------------------------------------------------

def build_plans(edge_index):
    src = np.concatenate([edge_index[0], np.arange(N, dtype=np.int64)]).astype(np.int64)
    dst = np.concatenate([edge_index[1], np.arange(N, dtype=np.int64)]).astype(np.int64)
    plans = []
    for c in range(SH):
        m = (dst >= c * NS) & (dst < (c + 1) * NS)
        s_c = src[m]
        d_c = dst[m] - c * NS
        passes = []
        for lo in (True, False):
            pm = (s_c < SPLIT) if lo else (s_c >= SPLIT)
            s_p = s_c[pm]
            d_p = d_c[pm]
            deg = np.bincount(d_p, minlength=NS)
            order = np.argsort(-deg, kind="stable").astype(np.int32)
            rank = np.empty(NS, np.int32)
            rank[order] = np.arange(NS, dtype=np.int32)
            eo = np.argsort(rank[d_p].astype(np.int64), kind="stable")
            s_sorted = s_p[eo]
            deg_sorted = deg[order]
            J = np.array(
                [int(deg_sorted[b * 128:(b + 1) * 128].max()) if b * 128 < NS else 0
                 for b in range(NBLK)], np.int32)
            passes.append(dict(lo=lo, order=order, rank=rank, J=J,
                               s_sorted=s_sorted, deg_sorted=deg_sorted))
        plans.append(passes)

    for b in range(NBLK):
        for pi in range(2):
            Jm = max(int(plans[c][pi]["J"][b]) for c in range(SH))
            for c in range(SH):
                plans[c][pi]["J"][b] = Jm

    for c in range(SH):
        for pi in range(2):
            pl = plans[c][pi]
            lo = pl["lo"]
            dummy = 0 if lo else HI_DUMMY
            starts = np.zeros(NS + 1, np.int64)
            np.cumsum(pl["deg_sorted"], out=starts[1:])
            idx_blocks = []
            for b in range(NBLK):
                J = int(pl["J"][b])
                if J == 0:
                    idx_blocks.append(np.zeros((0,), np.int16))
                    continue
                grid = np.full((128, J), dummy, np.int64)
                nrows = min(128, NS - b * 128)
                for p in range(nrows):
                    r = b * 128 + p
                    d0, d1 = starts[r], starts[r + 1]
                    sv = pl["s_sorted"][d0:d1]
                    grid[p, : d1 - d0] = (sv + 1) if lo else (sv - SPLIT)
                idx_blocks.append(grid.T.reshape(-1).astype(np.int16))
            pl["idx_blocks"] = idx_blocks
    return plans


def pack_idx16(idx):
    n = len(idx)
    a = idx.reshape(n // 16, 16).T
    return np.tile(a, (8, 1))


def host_meta(plans):
    metas = []
    for c in range(SH):
        meta = {}
        for pi, tag in ((0, "L"), (1, "H")):
            pl = plans[c][pi]
            cols = [pack_idx16(ib) for ib in pl["idx_blocks"] if len(ib)]
            meta[f"idx{tag}"] = (np.concatenate(cols, axis=1) if cols
                                 else np.zeros((128, 16), np.int16))
            gids = np.minimum(pl["order"].astype(np.int64) + c * NS, NPAD - 1)
            pad = np.full(NBLK * 128 - NS, NPAD - 1, np.int64)
            meta[f"gid{tag}"] = np.concatenate([gids, pad]).astype(np.int32)  # [NBLK*128]
        pl_L, pl_H = plans[c][0], plans[c][1]
        hrow = pl_L["rank"][pl_H["order"]].astype(np.int32)
        pad = np.arange(NS, NBLK * 128, dtype=np.int32)
        meta["h_align"] = np.concatenate([hrow, pad]).reshape(NBLK, 128).T.copy()  # [128, NBLK]
        gl = meta["gidL"].reshape(NBLK, 128).T
        gh = meta["gidH"].reshape(NBLK, 128).T
        meta["adst_gidx"] = np.concatenate([gl, gh], axis=1).astype(np.int32)  # [128, 2*NBLK]
        metas.append(meta)
    return metas


# --------------------------------------------------------------------------
# shared device helpers
# --------------------------------------------------------------------------

EMIT_LVL = 4


def emit_pass_blocks(nc, pools, tabs, meta_sb, Jlist, pass_idx, nheads, ch, rowe,
                     out_plain, out_scatter, jcap):
    lvl = EMIT_LVL
    """Emit one aggregation pass (all blocks) of one layer.

    meta_sb: dict with idx DRAM tensors [idxL, idxH] and SBUF APs
    {kneg: [128, 2*NBLK*nheads], adst: [128, 2*NBLK*nheads],
    halign: [128, NBLK]}. Blocks with J > jcap are processed in slot chunks
    accumulated into P.
    """
    hcw = nheads * ch
    lo = pass_idx == 0
    tab = tabs[0] if lo else tabs[1]
    idx_dram = meta_sb["idx"][pass_idx]
    off = 0
    for b in range(NBLK):
        J = int(Jlist[b])
        pbi = pass_idx * NBLK + b
        P = pools["pp"].tile([128, hcw + nheads], F32, tag="ptile")
        if J == 0:
            nc.vector.memset(P[:], 0.0)
        if J > 0:
            idxs = pools["ip"].tile([128, 8 * J], I16, tag="idx")
            nc.sync.dma_start(out=idxs[:], in_=idx_dram[:, off:off + 8 * J])
        for j0 in range(0, J, jcap):
            Jc = min(jcap, J - j0)
            G = pools["gp"].tile([128, Jc, rowe], FP16, tag="gtile")
            nc.gpsimd.dma_gather(
                out_ap=G[:, :, :],
                in_ap=tab[:, :],
                idxs_ap=idxs[:, 8 * j0:8 * (j0 + Jc)],
                num_idxs=Jc * 128,
                num_idxs_reg=Jc * 128,
                elem_size=rowe,
                single_packet=False,
            )
            if lvl < 2:
                continue
            alpha = pools["ap"].tile([128, Jc, nheads], F32, tag="alpha")
            nc.vector.tensor_tensor(
                out=alpha[:],
                in0=G[:, :, hcw:hcw + nheads],
                in1=meta_sb["adst"][:, pbi * nheads:(pbi + 1) * nheads]
                    .rearrange("p (j h) -> p j h", j=1)
                    .to_broadcast([128, Jc, nheads]),
                op=ALU.add,
            )
            asl = pools["ap"].tile([128, Jc, nheads], F32, tag="asl")
            nc.vector.tensor_scalar_mul(asl[:], alpha[:], SLOPE)
            nc.vector.tensor_tensor(out=alpha[:], in0=alpha[:], in1=asl[:],
                                    op=ALU.max)
            if lvl < 3:
                continue
            E = pools["ep"].tile([128, Jc, hcw], FP16, tag="etile")
            den = pools["ap"].tile([128, nheads], F32, tag="den")
            for h in range(nheads):
                nc.scalar.activation(
                    out=E[:, :, h * ch:(h + 1) * ch],
                    in_=alpha[:, :, h:h + 1].to_broadcast([128, Jc, ch]),
                    func=AF.Exp,
                    bias=meta_sb["kneg"][:, pbi * nheads + h:pbi * nheads + h + 1],
                    accum_out=den[:, h:h + 1],
                )
            if j0 == 0:
                nc.vector.tensor_copy(out=P[:, hcw:], in_=den[:])
            else:
                nc.vector.tensor_tensor(out=P[:, hcw:], in0=P[:, hcw:],
                                        in1=den[:], op=ALU.add)
            if lvl < 4:
                continue
            M = pools["mp"].tile([128, Jc, hcw], FP16, tag="mtile")
            nc.vector.tensor_tensor(out=M[:], in0=G[:, :, 0:hcw], in1=E[:],
                                    op=ALU.mult)
            # pairwise tree sum over slots, ping-ponging between M and E
            cur, nxt, k = M, E, Jc
            while k > 1:
                k2 = k // 2
                half = k - k2
                nc.vector.tensor_tensor(out=nxt[:, 0:k2, :], in0=cur[:, 0:k2, :],
                                        in1=cur[:, half:half + k2, :], op=ALU.add)
                if k % 2:
                    nc.vector.tensor_copy(out=nxt[:, k2:k2 + 1, :],
                                          in_=cur[:, k2:k2 + 1, :])
                cur, nxt = nxt, cur
                k = half
            if j0 == 0:
                nc.vector.tensor_copy(
                    out=P[:, 0:hcw],
                    in_=cur[:, 0:1, :].rearrange("p j r -> p (j r)"))
            else:
                nc.vector.tensor_tensor(
                    out=P[:, 0:hcw], in0=P[:, 0:hcw],
                    in1=cur[:, 0:1, :].rearrange("p j r -> p (j r)"),
                    op=ALU.add,
                )
        off += 8 * J
        if lo:
            nc.sync.dma_start(out=out_plain[b * 128:(b + 1) * 128, :], in_=P[:])
        else:
            nc.gpsimd.indirect_dma_start(
                out=out_scatter[:, :],
                out_offset=IndirectOffsetOnAxis(
                    ap=meta_sb["halign"][:, b:b + 1], axis=0),
                in_=P[:],
                in_offset=None,
            )


# --------------------------------------------------------------------------
# program 1: phase1 (tables) + layer-1 aggregation + combine + h2 matmul
# --------------------------------------------------------------------------

def build_prog1(JL, JH, CL, CH, stages=5):
    nc = bacc.Bacc("TRN2", target_bir_lowering=False, debug=False)
    xT = nc.declare_dram_parameter("xT", [F_IN, NPAD], FP16, isOutput=False)
    w1e = nc.declare_dram_parameter("w1ext", [F_IN, HC + 8], FP16, isOutput=False)
    w2e = nc.declare_dram_parameter("w2ext", [HC, NCLS + 2], FP16, isOutput=False)
    maxa = nc.declare_dram_parameter("maxasrc", [128, H], F32, isOutput=False)
    b1r = nc.declare_dram_parameter("b1rep", [128, HC], F32, isOutput=False)
    idxL = nc.declare_dram_parameter("idxL", [128, CL], I16, isOutput=False)
    idxH = nc.declare_dram_parameter("idxH", [128, CH], I16, isOutput=False)
    gidx = nc.declare_dram_parameter("adst_gidx", [128, 2 * NBLK], I32, isOutput=False)
    halign = nc.declare_dram_parameter("halign", [128, NBLK], I32, isOutput=False)
    h2a = nc.declare_dram_parameter("h2a", [NBLK * 128, NCLS + 2], F32, isOutput=True)

    T_lo = nc.dram_tensor("T_lo", [LO_ROWS, ROW1], FP16)
    T_hi = nc.dram_tensor("T_hi", [HI_ROWS, ROW1], FP16)
    adst = nc.dram_tensor("adst", [NPAD, H], FP16)
    PL = nc.dram_tensor("PL", [NBLK * 128, RW], F32)
    PHA = nc.dram_tensor("PHA", [NBLK * 128, RW], F32)

    with TileContext(nc) as tc:
        with (
            tc.tile_pool(name="const", bufs=1) as cp,
            tc.tile_pool(name="psum", bufs=2, space="PSUM") as psp,
        ):
            nc.gpsimd.load_library(library_config.mlp)
            # ---- phase 1: build node tables ----
            phase1 = (tc.tile_pool(name="xslab", bufs=2),
                      tc.tile_pool(name="rows", bufs=2))
            xp, rp = phase1[0].__enter__(), phase1[1].__enter__()
            w1sb = cp.tile([128, 2 * (HC + 8)], FP16)
            nc.sync.dma_start(out=w1sb[:, 0:HC + 8], in_=w1e[0:128, :])
            nc.sync.dma_start(out=w1sb[:, HC + 8:], in_=w1e[128:256, :])
            dummy = cp.tile([1, ROW1], FP16)
            nc.vector.memset(dummy[:], 0.0)
            nc.vector.memset(dummy[:, HC:HC + H], ASRC_DUMMY)
            nc.sync.dma_start(out=T_lo[0:1, :], in_=dummy[:])
            nc.sync.dma_start(out=T_hi[HI_DUMMY:HI_DUMMY + 1, :], in_=dummy[:])

            SW = SLAB * 128
            for s in range(NSLAB):
                n0 = s * SW
                xs = xp.tile([128, 2 * SW], FP16, tag="xs")
                nc.sync.dma_start(out=xs[:, 0:SW], in_=xT[0:128, n0:n0 + SW])
                nc.sync.dma_start(out=xs[:, SW:], in_=xT[128:256, n0:n0 + SW])
                rows = rp.tile([128, SLAB, HC + 2 * H], FP16, tag="rows")
                for bb in range(SLAB):
                    ps = psp.tile([128, HC + 8], F32, tag="mm1")
                    for k in range(2):
                        nc.tensor.matmul(
                            out=ps[:],
                            lhsT=xs[:, k * SW + bb * 128:k * SW + (bb + 1) * 128],
                            rhs=w1sb[:, k * (HC + 8):(k + 1) * (HC + 8)],
                            start=(k == 0),
                            stop=(k == 1),
                        )
                    nc.scalar.activation(
                        out=rows[:, bb:bb + 1, :].rearrange("p j r -> p (j r)"),
                        in_=ps[:, 0:HC + 2 * H], func=AF.Copy)
                nc.sync.dma_start(
                    out=adst[n0:n0 + SW, :].rearrange("(b p) h -> p b h", p=128),
                    in_=rows[:, :, RW:HC + 2 * H],
                )
                lo_end = SPLIT - n0   # nodes with slab-local id < lo_end go to T_lo
                if lo_end >= SW:
                    nc.sync.dma_start(
                        out=T_lo[n0 + 1:n0 + 1 + SW, 0:RW]
                            .rearrange("(b p) r -> p b r", p=128),
                        in_=rows[:, :, 0:RW],
                    )
                elif lo_end <= 0:
                    r0 = n0 - SPLIT
                    nc.sync.dma_start(
                        out=T_hi[r0:r0 + SW, 0:RW]
                            .rearrange("(b p) r -> p b r", p=128),
                        in_=rows[:, :, 0:RW],
                    )
                else:
                    bfull = lo_end // 128
                    prem = lo_end - bfull * 128
                    if bfull:
                        nc.sync.dma_start(
                            out=T_lo[n0 + 1:n0 + 1 + bfull * 128, 0:RW]
                                .rearrange("(b p) r -> p b r", p=128),
                            in_=rows[:, 0:bfull, 0:RW],
                        )
                    if prem:
                        nc.sync.dma_start(
                            out=T_lo[n0 + 1 + bfull * 128:n0 + 1 + lo_end, 0:RW]
                                .rearrange("(b p) r -> p b r", p=prem),
                            in_=rows[0:prem, bfull:bfull + 1, 0:RW],
                        )
                    nc.sync.dma_start(
                        out=T_hi[0:128 - prem, 0:RW]
                            .rearrange("(b p) r -> p b r", p=128 - prem),
                        in_=rows[prem:128, bfull:bfull + 1, 0:RW],
                    )
                    nrem = SLAB - bfull - 1
                    if nrem:
                        nc.sync.dma_start(
                            out=T_hi[128 - prem:128 - prem + nrem * 128, 0:RW]
                                .rearrange("(b p) r -> p b r", p=128),
                            in_=rows[:, bfull + 1:, 0:RW],
                        )
            for p in reversed(phase1):
                p.__exit__(None, None, None)

            if stages >= 2:
                # ---- phase 1.5: per-block a_dst + kneg ----
                maxasb = cp.tile([128, H], F32)
                nc.sync.dma_start(out=maxasb[:], in_=maxa[:, :])
                gsb = cp.tile([128, 2 * NBLK], I32)
                nc.sync.dma_start(out=gsb[:], in_=gidx[:, :])
                hasb = cp.tile([128, NBLK], I32)
                nc.sync.dma_start(out=hasb[:], in_=halign[:, :])
                adsts = cp.tile([128, 2 * NBLK * H], FP16)
                for k in range(2 * NBLK):
                    nc.gpsimd.indirect_dma_start(
                        out=adsts[:, k * H:(k + 1) * H],
                        out_offset=None,
                        in_=adst[:, :],
                        in_offset=IndirectOffsetOnAxis(ap=gsb[:, k:k + 1], axis=0),
                    )
                knegs = cp.tile([128, 2 * NBLK * H], F32)
                nc.vector.tensor_tensor(
                    out=knegs[:].rearrange("p (b h) -> p b h", h=H),
                    in0=adsts[:].rearrange("p (b h) -> p b h", h=H),
                    in1=maxasb[:].rearrange("p (j h) -> p j h", j=1)
                        .to_broadcast([128, 2 * NBLK, H]),
                    op=ALU.add,
                )
                ksl = cp.tile([128, 2 * NBLK * H], F32)
                nc.vector.tensor_scalar_mul(ksl[:], knegs[:], SLOPE)
                nc.vector.tensor_tensor(out=knegs[:], in0=knegs[:], in1=ksl[:],
                                        op=ALU.max)
                nc.vector.tensor_scalar_mul(knegs[:], knegs[:], -1.0)

            if stages >= 3:
                # ---- phase 2: both passes ----
                phase2 = (tc.tile_pool(name="idxp", bufs=2),
                          tc.tile_pool(name="gath", bufs=2),
                          tc.tile_pool(name="attn", bufs=2),
                          tc.tile_pool(name="etile", bufs=2),
                          tc.tile_pool(name="mtile", bufs=2),
                          tc.tile_pool(name="ptile", bufs=3),
                          tc.tile_pool(name="ph3", bufs=2))
                ip, gp, apool, ep, mp, pp, p3 = (p.__enter__() for p in phase2)
                pools = dict(ip=ip, gp=gp, ap=apool, ep=ep, mp=mp, pp=pp)
                meta_sb = dict(idx=[idxL, idxH], kneg=knegs[:],
                               halign=hasb[:], adst=adsts[:])
                emit_pass_blocks(nc, pools, (T_lo, T_hi), meta_sb, JL, 0, H, C,
                                 ROW1, PL, PHA, jcap=32)
                if stages >= 4:
                    emit_pass_blocks(nc, pools, (T_lo, T_hi), meta_sb, JH, 1, H,
                                     C, ROW1, PL, PHA, jcap=32)

            if stages >= 5:
                # ---- phase 3: combine + elu + h2 ----
                b1sb = cp.tile([128, HC], F32)
                nc.sync.dma_start(out=b1sb[:], in_=b1r[:, :])
                w2sb = cp.tile([128, 2 * (NCLS + 2)], FP16)
                nc.sync.dma_start(out=w2sb[:, 0:NCLS + 2], in_=w2e[0:128, :])
                nc.sync.dma_start(out=w2sb[:, NCLS + 2:], in_=w2e[128:256, :])
                ident = cp.tile([128, 128], FP16)
                make_identity(nc, ident[:])
                for b in range(NBLK):
                    sl = p3.tile([128, RW], F32, tag="sl")
                    sh = p3.tile([128, RW], F32, tag="sh")
                    nc.sync.dma_start(out=sl[:], in_=PL[b * 128:(b + 1) * 128, :])
                    nc.sync.dma_start(out=sh[:], in_=PHA[b * 128:(b + 1) * 128, :])
                    nc.vector.tensor_tensor(out=sl[:], in0=sl[:], in1=sh[:],
                                            op=ALU.add)
                    nc.vector.tensor_scalar_add(sl[:, HC:HC + H],
                                                sl[:, HC:HC + H], 1e-12)
                    rec = p3.tile([128, H, 1], F32, tag="rec")
                    nc.vector.reciprocal(
                        rec[:, :, 0:1].rearrange("p h j -> p (h j)"),
                        sl[:, HC:HC + H])
                    nc.vector.tensor_scalar_mul(rec[:], rec[:], float(C))
                    o = p3.tile([128, HC], F32, tag="o")
                    nc.vector.tensor_tensor(
                        out=o[:].rearrange("p (h c) -> p h c", h=H),
                        in0=sl[:, 0:HC].rearrange("p (h c) -> p h c", h=H),
                        in1=rec[:].to_broadcast([128, H, C]),
                        op=ALU.mult,
                    )
                    nc.vector.tensor_tensor(out=o[:], in0=o[:], in1=b1sb[:],
                                            op=ALU.add)
                    # elu(o) = relu(o) + exp(min(o,0)) - 1
                    pos = p3.tile([128, HC], F32, tag="pos")
                    nc.scalar.activation(out=pos[:], in_=o[:], func=AF.Relu)
                    nc.vector.tensor_scalar_min(o[:], o[:], 0.0)
                    nc.scalar.activation(out=o[:], in_=o[:], func=AF.Exp)
                    nc.vector.tensor_tensor(out=o[:], in0=o[:], in1=pos[:],
                                            op=ALU.add)
                    elu = p3.tile([128, HC], FP16, tag="elu")
                    nc.vector.tensor_scalar_add(elu[:], o[:], -1.0)
                    ps2 = psp.tile([128, NCLS + 2], F32, tag="mm2")
                    for k in range(2):
                        pst = psp.tile([128, 128], FP16, tag="ptr")
                        nc.tensor.transpose(out=pst[:],
                                            in_=elu[:, k * 128:(k + 1) * 128],
                                            identity=ident[:])
                        eT = p3.tile([128, 128], FP16, tag="eT")
                        nc.vector.tensor_copy(out=eT[:], in_=pst[:])
                        nc.tensor.matmul(
                            out=ps2[:], lhsT=eT[:],
                            rhs=w2sb[:, k * (NCLS + 2):(k + 1) * (NCLS + 2)],
                            start=(k == 0), stop=(k == 1))
                    h2sb = p3.tile([128, NCLS + 2], F32, tag="h2sb")
                    nc.vector.tensor_copy(out=h2sb[:], in_=ps2[:])
                    nc.sync.dma_start(out=h2a[b * 128:(b + 1) * 128, :],
                                      in_=h2sb[:])
            else:
                zz = cp.tile([128, NCLS + 2], F32)
                nc.vector.memset(zz[:], 0.0)
                for b in range(NBLK):
                    nc.sync.dma_start(out=h2a[b * 128:(b + 1) * 128, :],
                                      in_=zz[:])
            if stages >= 3:
                for p in reversed(phase2):
                    p.__exit__(None, None, None)
    nc.compile()
    return nc


# --------------------------------------------------------------------------
# program 2: layer-2 aggregation + output
# --------------------------------------------------------------------------

def build_prog2(JL, JH, CL, CH):
    nc = bacc.Bacc("TRN2", target_bir_lowering=False, debug=False)
    t2lo = nc.declare_dram_parameter("T2_lo", [LO_ROWS, ROW2], FP16, isOutput=False)
    t2hi = nc.declare_dram_parameter("T2_hi", [HI_ROWS, ROW2], FP16, isOutput=False)
    idxL = nc.declare_dram_parameter("idxL", [128, CL], I16, isOutput=False)
    idxH = nc.declare_dram_parameter("idxH", [128, CH], I16, isOutput=False)
    kneg2 = nc.declare_dram_parameter("kneg2", [128, 2 * NBLK], F32, isOutput=False)
    adst2 = nc.declare_dram_parameter("adst2", [128, 2 * NBLK], F32, isOutput=False)
    halign = nc.declare_dram_parameter("halign", [128, NBLK], I32, isOutput=False)
    b2r = nc.declare_dram_parameter("b2rep", [128, NCLS], F32, isOutput=False)
    out2 = nc.declare_dram_parameter("out2", [NBLK * 128, NCLS], F32, isOutput=True)

    PL = nc.dram_tensor("PL2", [NBLK * 128, NCLS + 1], F32)
    PHA = nc.dram_tensor("PHA2", [NBLK * 128, NCLS + 1], F32)

    with TileContext(nc) as tc:
        with (
            tc.tile_pool(name="const", bufs=1) as cp,
            tc.tile_pool(name="idxp", bufs=2) as ip,
            tc.tile_pool(name="gath", bufs=3) as gp,
            tc.tile_pool(name="attn", bufs=2) as apool,
            tc.tile_pool(name="etile", bufs=2) as ep,
            tc.tile_pool(name="mtile", bufs=2) as mp,
            tc.tile_pool(name="ptile", bufs=3) as pp,
            tc.tile_pool(name="ph3", bufs=2) as p3,
        ):
            nc.gpsimd.load_library(library_config.mlp)
            knsb = cp.tile([128, 2 * NBLK], F32)
            nc.sync.dma_start(out=knsb[:], in_=kneg2[:, :])
            adsb = cp.tile([128, 2 * NBLK], F32)
            nc.sync.dma_start(out=adsb[:], in_=adst2[:, :])
            hasb = cp.tile([128, NBLK], I32)
            nc.sync.dma_start(out=hasb[:], in_=halign[:, :])
            pools = dict(ip=ip, gp=gp, ap=apool, ep=ep, mp=mp, pp=pp)
            meta_sb = dict(idx=[idxL, idxH], kneg=knsb[:],
                           halign=hasb[:], adst=adsb[:])
            emit_pass_blocks(nc, pools, (t2lo, t2hi), meta_sb, JL, 0, 1, NCLS, ROW2,
                             PL, PHA, jcap=64)
            emit_pass_blocks(nc, pools, (t2lo, t2hi), meta_sb, JH, 1, 1, NCLS, ROW2,
                             PL, PHA, jcap=64)

            b2sb = cp.tile([128, NCLS], F32)
            nc.sync.dma_start(out=b2sb[:], in_=b2r[:, :])
            for b in range(NBLK):
                sl = p3.tile([128, NCLS + 1], F32, tag="sl")
                sh = p3.tile([128, NCLS + 1], F32, tag="sh")
                nc.sync.dma_start(out=sl[:], in_=PL[b * 128:(b + 1) * 128, :])
                nc.sync.dma_start(out=sh[:], in_=PHA[b * 128:(b + 1) * 128, :])
                nc.vector.tensor_tensor(out=sl[:], in0=sl[:], in1=sh[:], op=ALU.add)
                nc.vector.tensor_scalar_add(sl[:, NCLS:NCLS + 1],
                                            sl[:, NCLS:NCLS + 1], 1e-12)
                rec = p3.tile([128, 1], F32, tag="rec")
                nc.vector.reciprocal(rec[:], sl[:, NCLS:NCLS + 1])
                nc.vector.tensor_scalar_mul(rec[:], rec[:], float(NCLS))
                o = p3.tile([128, NCLS], F32, tag="o")
                nc.vector.tensor_tensor(
                    out=o[:].rearrange("p (j c) -> p j c", j=1),
                    in0=sl[:, 0:NCLS].rearrange("p (j c) -> p j c", j=1),
                    in1=rec[:].rearrange("p (j c) -> p j c", j=1)
                        .to_broadcast([128, 1, NCLS]),
                    op=ALU.mult,
                )
                nc.vector.tensor_tensor(out=o[:], in0=o[:], in1=b2sb[:], op=ALU.add)
                nc.sync.dma_start(out=out2[b * 128:(b + 1) * 128, :], in_=o[:])
    nc.compile()
    return nc


# --------------------------------------------------------------------------
# host glue
# --------------------------------------------------------------------------

LAST_RESULTS = []


def kernel(x, edge_index, W1, att_src1, att_dst1, b1, W2, att_src2, att_dst2, b2,
           **_):
    LAST_RESULTS.clear()
    x = np.asarray(x, np.float32)
    edge_index = np.asarray(edge_index)
    plans = build_plans(edge_index)
    metas = host_meta(plans)
    JL = plans[0][0]["J"]
    JH = plans[0][1]["J"]
    CL = max(8 * int(JL.sum()), 16)
    CH = max(8 * int(JH.sum()), 16)

    W1 = np.asarray(W1, np.float32)
    W1a = np.einsum("fhc,hc->fh", W1.reshape(F_IN, H, C),
                    np.asarray(att_src1, np.float32))
    W1b = np.einsum("fhc,hc->fh", W1.reshape(F_IN, H, C),
                    np.asarray(att_dst1, np.float32))
    w1ext = np.concatenate([W1, W1a, W1b], axis=1).astype(np.float16)
    xT = np.zeros((F_IN, NPAD), np.float16)
    xT[:, :N] = x.T.astype(np.float16)
    a_src_host = x @ W1a
    maxasrc = np.tile((a_src_host.max(axis=0) + 0.05).astype(np.float32)[None, :],
                      (128, 1))

    W2 = np.asarray(W2, np.float32)
    W2a = W2 @ np.asarray(att_src2, np.float32).reshape(NCLS, 1)
    W2b = W2 @ np.asarray(att_dst2, np.float32).reshape(NCLS, 1)
    w2ext = np.concatenate([W2, W2a, W2b], axis=1).astype(np.float16)
    b1rep = np.tile(np.asarray(b1, np.float32)[None, :], (128, 1))
    b2rep = np.tile(np.asarray(b2, np.float32)[None, :], (128, 1))

    nc1 = build_prog1(JL, JH, CL, CH)
    in_maps = []
    for c in range(SH):
        m = metas[c]
        in_maps.append(dict(
            xT=xT, w1ext=w1ext, w2ext=w2ext, maxasrc=maxasrc, b1rep=b1rep,
            idxL=np.ascontiguousarray(m["idxL"]),
            idxH=np.ascontiguousarray(m["idxH"]),
            adst_gidx=np.ascontiguousarray(m["adst_gidx"]),
            halign=np.ascontiguousarray(m["h_align"]),
        ))
    res1 = run_bass_kernel_spmd(nc1, in_maps, core_ids=list(range(SH)))
    LAST_RESULTS.append(res1)

    # assemble full layer-2 node table on host
    h2_full = np.zeros((NPAD, NCLS + 2), np.float32)
    for c in range(SH):
        h2a = res1.results[c]["h2a"]
        order = plans[c][0]["order"].astype(np.int64)
        h2_full[order + c * NS] = h2a[:NS]
    rows2 = np.zeros((NPAD, ROW2), np.float16)
    rows2[:, :NCLS + 1] = h2_full[:, :NCLS + 1].astype(np.float16)
    T2_lo = np.zeros((LO_ROWS, ROW2), np.float16)
    T2_lo[1:] = rows2[:SPLIT]
    T2_lo[0, NCLS] = ASRC_DUMMY
    T2_hi = np.zeros((HI_ROWS, ROW2), np.float16)
    T2_hi[:HI_DUMMY] = rows2[SPLIT:]
    T2_hi[HI_DUMMY, NCLS] = ASRC_DUMMY
    a_dst2 = h2_full[:, NCLS + 1]
    max2 = float(h2_full[:N, NCLS].max()) + 0.05

    nc2 = build_prog2(JL, JH, CL, CH)
    in_maps2 = []
    for c in range(SH):
        m = metas[c]
        ad = a_dst2[np.concatenate([m["gidL"], m["gidH"]]).astype(np.int64)]
        ad = ad.reshape(2 * NBLK, 128).T.astype(np.float32)
        t = ad + max2
        kneg = -np.maximum(t, SLOPE * t)
        in_maps2.append(dict(
            T2_lo=T2_lo, T2_hi=T2_hi,
            idxL=np.ascontiguousarray(m["idxL"]),
            idxH=np.ascontiguousarray(m["idxH"]),
            kneg2=np.ascontiguousarray(kneg),
            adst2=np.ascontiguousarray(ad),
            halign=np.ascontiguousarray(m["h_align"]),
            b2rep=b2rep,
        ))
    res2 = run_bass_kernel_spmd(nc2, in_maps2, core_ids=list(range(SH)))
    LAST_RESULTS.append(res2)

    out = np.zeros((N, NCLS), np.float32)
    for c in range(SH):
        o2 = res2.results[c]["out2"]
        order = plans[c][0]["order"].astype(np.int64)
        out[order + c * NS] = o2[:NS]
    return out


# revision 21
# speedup vs baseline: 1.0927x; 1.0927x over previous
"""Two-layer GAT on Trainium2 (8 NeuronCores, SPMD).

Strategy (graph/data parallel, dst-sharded):
- Nodes are sharded across 8 cores by contiguous destination ranges (6250 each).
- Phase 1 (replicated on every core): one fused matmul
  x @ [W1 | W1@att_src.T | W1@att_dst.T] produces per-node h, a_src, a_dst.
  Feature-table rows [h(256)|a_src(4)|pad] fp16 (768B) are written to two DRAM
  tables (lo: nodes < 32767, hi: rest) because the fast gather
  (InstDMAGatherAnt) takes int16 row indices.
- Phase 2: per core, edges (incl. self loops) grouped by dst, two passes by
  src range. In each pass the shard's dsts are sorted by pass-degree and
  packed into blocks of 128 (partition dim) x J[b] slots; padding points at a
  dummy table row whose a_src=-30000 so exp()==0. One dma_gather per block
  fetches all source rows; e=exp(lrelu(a_src+a_dst)+kneg) is expanded on the
  Scalar engine (which also emits the softmax denominator via accum_out, and
  kneg is a per-dst shift keeping exp() in fp16 range); messages are weighted
  on the Vector engine and pairwise-tree-summed over slots. Per-dst num|den
  partials go to DRAM (the H pass scatters into L-pass row order).
- Phase 3: combine passes, normalize, +bias, ELU, h2 = elu @ W2ext
  (transpose via TensorE). Per-node layer-1 results return to the host, which
  assembles the full layer-2 table (fp16, 256B rows) for launch 2.
- Launch 2 repeats phases 2/3 for the output layer (1 head, 40 classes).
"""
import sys

import numpy as np

sys.path.insert(0, "/opt/trn_rl_repo")

import concourse.bacc as bacc
import concourse.bass as bass
import concourse.mybir as mybir
from concourse import library_config
from concourse.bass import IndirectOffsetOnAxis
from concourse.bass_utils import run_bass_kernel_spmd
from concourse.masks import make_identity
from concourse.tile import TileContext

FP16 = mybir.dt.float16
F32 = mybir.dt.float32
I16 = mybir.dt.int16
I32 = mybir.dt.int32
AF = mybir.ActivationFunctionType
ALU = mybir.AluOpType

N = 50000
F_IN = 256
H = 4
C = 64
HC = H * C            # 256
NCLS = 40
SLOPE = 0.2
SH = 8
NS = N // SH          # 6250
NPAD = 50176          # 392 * 128
SPLIT = 32767         # nodes < SPLIT -> T_lo at row node+1 (row 0 = dummy)
LO_ROWS = 32768
HI_ROWS = NPAD - SPLIT + 1   # 17410 (last row = dummy)
HI_DUMMY = NPAD - SPLIT      # 17409
ROW1 = 384            # fp16 elems -> 768B
ROW2 = 128            # fp16 elems -> 256B
NBLK = (NS + 127) // 128     # 49
ASRC_DUMMY = -30000.0
SLAB = 28             # node blocks per phase-1 slab (392 = 14*28)
NSLAB = NPAD // (SLAB * 128)
RW = HC + H           # 260: table-row payload elems (layer 1)


# --------------------------------------------------------------------------
# host-side edge plan
# --------------------------------------------------------------------------

def build_plans(edge_index):
    src = np.concatenate([edge_index[0], np.arange(N, dtype=np.int64)]).astype(np.int64)
    dst = np.concatenate([edge_index[1], np.arange(N, dtype=np.int64)]).astype(np.int64)
    plans = []
    for c in range(SH):
        m = (dst >= c * NS) & (dst < (c + 1) * NS)
        s_c = src[m]
        d_c = dst[m] - c * NS
        passes = []
        for lo in (True, False):
            pm = (s_c < SPLIT) if lo else (s_c >= SPLIT)
            s_p = s_c[pm]
            d_p = d_c[pm]
            deg = np.bincount(d_p, minlength=NS)
            order = np.argsort(-deg, kind="stable").astype(np.int32)
            rank = np.empty(NS, np.int32)
            rank[order] = np.arange(NS, dtype=np.int32)
            eo = np.argsort(rank[d_p].astype(np.int64), kind="stable")
            s_sorted = s_p[eo]
            deg_sorted = deg[order]
            J = np.array(
                [int(deg_sorted[b * 128:(b + 1) * 128].max()) if b * 128 < NS else 0
                 for b in range(NBLK)], np.int32)
            passes.append(dict(lo=lo, order=order, rank=rank, J=J,
                               s_sorted=s_sorted, deg_sorted=deg_sorted))
        plans.append(passes)

    for b in range(NBLK):
        for pi in range(2):
            Jm = max(int(plans[c][pi]["J"][b]) for c in range(SH))
            for c in range(SH):
                plans[c][pi]["J"][b] = Jm

    for c in range(SH):
        for pi in range(2):
            pl = plans[c][pi]
            lo = pl["lo"]
            dummy = 0 if lo else HI_DUMMY
            starts = np.zeros(NS + 1, np.int64)
            np.cumsum(pl["deg_sorted"], out=starts[1:])
            idx_blocks = []
            for b in range(NBLK):
                J = int(pl["J"][b])
                if J == 0:
                    idx_blocks.append(np.zeros((0,), np.int16))
                    continue
                grid = np.full((128, J), dummy, np.int64)
                nrows = min(128, NS - b * 128)
                for p in range(nrows):
                    r = b * 128 + p
                    d0, d1 = starts[r], starts[r + 1]
                    sv = pl["s_sorted"][d0:d1]
                    grid[p, : d1 - d0] = (sv + 1) if lo else (sv - SPLIT)
                idx_blocks.append(grid.T.reshape(-1).astype(np.int16))
            pl["idx_blocks"] = idx_blocks
    return plans


def pack_idx16(idx):
    n = len(idx)
    a = idx.reshape(n // 16, 16).T
    return np.tile(a, (8, 1))


def host_meta(plans):
    metas = []
    for c in range(SH):
        meta = {}
        for pi, tag in ((0, "L"), (1, "H")):
            pl = plans[c][pi]
            cols = [pack_idx16(ib) for ib in pl["idx_blocks"] if len(ib)]
            meta[f"idx{tag}"] = (np.concatenate(cols, axis=1) if cols
                                 else np.zeros((128, 16), np.int16))
            gids = np.minimum(pl["order"].astype(np.int64) + c * NS, NPAD - 1)
            pad = np.full(NBLK * 128 - NS, NPAD - 1, np.int64)
            meta[f"gid{tag}"] = np.concatenate([gids, pad]).astype(np.int32)  # [NBLK*128]
        pl_L, pl_H = plans[c][0], plans[c][1]
        hrow = pl_L["rank"][pl_H["order"]].astype(np.int32)
        pad = np.arange(NS, NBLK * 128, dtype=np.int32)
        meta["h_align"] = np.concatenate([hrow, pad]).reshape(NBLK, 128).T.copy()  # [128, NBLK]
        gl = meta["gidL"].reshape(NBLK, 128).T
        gh = meta["gidH"].reshape(NBLK, 128).T
        meta["adst_gidx"] = np.concatenate([gl, gh], axis=1).astype(np.int32)  # [128, 2*NBLK]
        metas.append(meta)
    return metas


# --------------------------------------------------------------------------
# shared device helpers
# --------------------------------------------------------------------------

EMIT_LVL = 4


def emit_pass_blocks(nc, pools, tabs, meta_sb, Jlist, pass_idx, nheads, ch, rowe,
                     out_plain, out_scatter, jcap):
    lvl = EMIT_LVL
    """Emit one aggregation pass (all blocks) of one layer.

    meta_sb: dict with idx DRAM tensors [idxL, idxH] and SBUF APs
    {kneg: [128, 2*NBLK*nheads], adst: [128, 2*NBLK*nheads],
    halign: [128, NBLK]}. Blocks with J > jcap are processed in slot chunks
    accumulated into P.
    """
    hcw = nheads * ch
    lo = pass_idx == 0
    tab = tabs[0] if lo else tabs[1]
    idx_dram = meta_sb["idx"][pass_idx]
    off = 0
    for b in range(NBLK):
        J = int(Jlist[b])
        pbi = pass_idx * NBLK + b
        P = pools["pp"].tile([128, hcw + nheads], F32, tag="ptile")
        if J == 0:
            nc.vector.memset(P[:], 0.0)
        if J > 0:
            idxs = pools["ip"].tile([128, 8 * J], I16, tag="idx")
            nc.sync.dma_start(out=idxs[:], in_=idx_dram[:, off:off + 8 * J])
        for j0 in range(0, J, jcap):
            Jc = min(jcap, J - j0)
            G = pools["gp"].tile([128, Jc, rowe], FP16, tag="gtile")
            nc.gpsimd.dma_gather(
                out_ap=G[:, :, :],
                in_ap=tab[:, :],
                idxs_ap=idxs[:, 8 * j0:8 * (j0 + Jc)],
                num_idxs=Jc * 128,
                num_idxs_reg=Jc * 128,
                elem_size=rowe,
                single_packet=False,
            )
            if lvl < 2:
                continue
            alpha = pools["ap"].tile([128, Jc, nheads], F32, tag="alpha")
            nc.vector.tensor_tensor(
                out=alpha[:],
                in0=G[:, :, hcw:hcw + nheads],
                in1=meta_sb["adst"][:, pbi * nheads:(pbi + 1) * nheads]
                    .rearrange("p (j h) -> p j h", j=1)
                    .to_broadcast([128, Jc, nheads]),
                op=ALU.add,
            )
            asl = pools["ap"].tile([128, Jc, nheads], F32, tag="asl")
            nc.vector.tensor_scalar_mul(asl[:], alpha[:], SLOPE)
            nc.vector.tensor_tensor(out=alpha[:], in0=alpha[:], in1=asl[:],
                                    op=ALU.max)
            if lvl < 3:
                continue
            E = pools["ep"].tile([128, Jc, hcw], FP16, tag="etile")
            den = pools["ap"].tile([128, nheads], F32, tag="den")
            for h in range(nheads):
                nc.scalar.activation(
                    out=E[:, :, h * ch:(h + 1) * ch],
                    in_=alpha[:, :, h:h + 1].to_broadcast([128, Jc, ch]),
                    func=AF.Exp,
                    bias=meta_sb["kneg"][:, pbi * nheads + h:pbi * nheads + h + 1],
                    accum_out=den[:, h:h + 1],
                )
            if j0 == 0:
                nc.vector.tensor_copy(out=P[:, hcw:], in_=den[:])
            else:
                nc.vector.tensor_tensor(out=P[:, hcw:], in0=P[:, hcw:],
                                        in1=den[:], op=ALU.add)
            if lvl < 4:
                continue
            M = pools["mp"].tile([128, Jc, hcw], FP16, tag="mtile")
            nc.vector.tensor_tensor(out=M[:], in0=G[:, :, 0:hcw], in1=E[:],
                                    op=ALU.mult)
            # pairwise tree sum over slots, ping-ponging between M and E
            cur, nxt, k = M, E, Jc
            while k > 1:
                k2 = k // 2
                half = k - k2
                nc.vector.tensor_tensor(out=nxt[:, 0:k2, :], in0=cur[:, 0:k2, :],
                                        in1=cur[:, half:half + k2, :], op=ALU.add)
                if k % 2:
                    nc.vector.tensor_copy(out=nxt[:, k2:k2 + 1, :],
                                          in_=cur[:, k2:k2 + 1, :])
                cur, nxt = nxt, cur
                k = half
            if j0 == 0:
                nc.vector.tensor_copy(
                    out=P[:, 0:hcw],
                    in_=cur[:, 0:1, :].rearrange("p j r -> p (j r)"))
            else:
                nc.vector.tensor_tensor(
                    out=P[:, 0:hcw], in0=P[:, 0:hcw],
                    in1=cur[:, 0:1, :].rearrange("p j r -> p (j r)"),
                    op=ALU.add,
                )
        off += 8 * J
        if lo:
            nc.sync.dma_start(out=out_plain[b * 128:(b + 1) * 128, :], in_=P[:])
        else:
            nc.gpsimd.indirect_dma_start(
                out=out_scatter[:, :],
                out_offset=IndirectOffsetOnAxis(
                    ap=meta_sb["halign"][:, b:b + 1], axis=0),
                in_=P[:],
                in_offset=None,
            )


# --------------------------------------------------------------------------
# program 1: phase1 (tables) + layer-1 aggregation + combine + h2 matmul
# --------------------------------------------------------------------------

def build_prog1(JL, JH, CL, CH, stages=5):
    nc = bacc.Bacc("TRN2", target_bir_lowering=False, debug=False)
    xT = nc.declare_dram_parameter("xT", [F_IN, NPAD], FP16, isOutput=False)
    w1e = nc.declare_dram_parameter("w1ext", [F_IN, HC + 8], FP16, isOutput=False)
    w2e = nc.declare_dram_parameter("w2ext", [HC, NCLS + 2], FP16, isOutput=False)
    maxa = nc.declare_dram_parameter("maxasrc", [128, H], F32, isOutput=False)
    b1r = nc.declare_dram_parameter("b1rep", [128, HC], F32, isOutput=False)
    idxL = nc.declare_dram_parameter("idxL", [128, CL], I16, isOutput=False)
    idxH = nc.declare_dram_parameter("idxH", [128, CH], I16, isOutput=False)
    gidx = nc.declare_dram_parameter("adst_gidx", [128, 2 * NBLK], I32, isOutput=False)
    halign = nc.declare_dram_parameter("halign", [128, NBLK], I32, isOutput=False)
    h2a = nc.declare_dram_parameter("h2a", [NBLK * 128, NCLS + 2], F32, isOutput=True)

    T_lo = nc.dram_tensor("T_lo", [LO_ROWS, ROW1], FP16)
    T_hi = nc.dram_tensor("T_hi", [HI_ROWS, ROW1], FP16)
    adst = nc.dram_tensor("adst", [NPAD, H], FP16)
    PL = nc.dram_tensor("PL", [NBLK * 128, RW], F32)
    PHA = nc.dram_tensor("PHA", [NBLK * 128, RW], F32)

    with TileContext(nc) as tc:
        with (
            tc.tile_pool(name="const", bufs=1) as cp,
            tc.tile_pool(name="psum", bufs=2, space="PSUM") as psp,
        ):
            nc.gpsimd.load_library(library_config.mlp)
            # ---- phase 1: build node tables ----
            phase1 = (tc.tile_pool(name="xslab", bufs=2),
                      tc.tile_pool(name="rows", bufs=2))
            xp, rp = phase1[0].__enter__(), phase1[1].__enter__()
            w1sb = cp.tile([128, 2 * (HC + 8)], FP16)
            nc.sync.dma_start(out=w1sb[:, 0:HC + 8], in_=w1e[0:128, :])
            nc.sync.dma_start(out=w1sb[:, HC + 8:], in_=w1e[128:256, :])
            dummy = cp.tile([1, ROW1], FP16)
            nc.vector.memset(dummy[:], 0.0)
            nc.vector.memset(dummy[:, HC:HC + H], ASRC_DUMMY)
            nc.sync.dma_start(out=T_lo[0:1, :], in_=dummy[:])
            nc.sync.dma_start(out=T_hi[HI_DUMMY:HI_DUMMY + 1, :], in_=dummy[:])

            SW = SLAB * 128
            for s in range(NSLAB):
                n0 = s * SW
                xs = xp.tile([128, 2 * SW], FP16, tag="xs")
                nc.sync.dma_start(out=xs[:, 0:SW], in_=xT[0:128, n0:n0 + SW])
                nc.sync.dma_start(out=xs[:, SW:], in_=xT[128:256, n0:n0 + SW])
                rows = rp.tile([128, SLAB, HC + 2 * H], FP16, tag="rows")
                for bb in range(SLAB):
                    ps = psp.tile([128, HC + 8], F32, tag="mm1")
                    for k in range(2):
                        nc.tensor.matmul(
                            out=ps[:],
                            lhsT=xs[:, k * SW + bb * 128:k * SW + (bb + 1) * 128],
                            rhs=w1sb[:, k * (HC + 8):(k + 1) * (HC + 8)],
                            start=(k == 0),
                            stop=(k == 1),
                        )
                    nc.scalar.activation(
                        out=rows[:, bb:bb + 1, :].rearrange("p j r -> p (j r)"),
                        in_=ps[:, 0:HC + 2 * H], func=AF.Copy)
                nc.sync.dma_start(
                    out=adst[n0:n0 + SW, :].rearrange("(b p) h -> p b h", p=128),
                    in_=rows[:, :, RW:HC + 2 * H],
                )
                lo_end = SPLIT - n0   # nodes with slab-local id < lo_end go to T_lo
                if lo_end >= SW:
                    nc.sync.dma_start(
                        out=T_lo[n0 + 1:n0 + 1 + SW, 0:RW]
                            .rearrange("(b p) r -> p b r", p=128),
                        in_=rows[:, :, 0:RW],
                    )
                elif lo_end <= 0:
                    r0 = n0 - SPLIT
                    nc.sync.dma_start(
                        out=T_hi[r0:r0 + SW, 0:RW]
                            .rearrange("(b p) r -> p b r", p=128),
                        in_=rows[:, :, 0:RW],
                    )
                else:
                    bfull = lo_end // 128
                    prem = lo_end - bfull * 128
                    if bfull:
                        nc.sync.dma_start(
                            out=T_lo[n0 + 1:n0 + 1 + bfull * 128, 0:RW]
                                .rearrange("(b p) r -> p b r", p=128),
                            in_=rows[:, 0:bfull, 0:RW],
                        )
                    if prem:
                        nc.sync.dma_start(
                            out=T_lo[n0 + 1 + bfull * 128:n0 + 1 + lo_end, 0:RW]
                                .rearrange("(b p) r -> p b r", p=prem),
                            in_=rows[0:prem, bfull:bfull + 1, 0:RW],
                        )
                    nc.sync.dma_start(
                        out=T_hi[0:128 - prem, 0:RW]
                            .rearrange("(b p) r -> p b r", p=128 - prem),
                        in_=rows[prem:128, bfull:bfull + 1, 0:RW],
                    )
                    nrem = SLAB - bfull - 1
                    if nrem:
                        nc.sync.dma_start(
                            out=T_hi[128 - prem:128 - prem + nrem * 128, 0:RW]
                                .rearrange("(b p) r -> p b r", p=128),
                            in_=rows[:, bfull + 1:, 0:RW],
                        )
            for p in reversed(phase1):
                p.__exit__(None, None, None)

            if stages >= 2:
                # ---- phase 1.5: per-block a_dst + kneg ----
                maxasb = cp.tile([128, H], F32)
                nc.sync.dma_start(out=maxasb[:], in_=maxa[:, :])
                gsb = cp.tile([128, 2 * NBLK], I32)
                nc.sync.dma_start(out=gsb[:], in_=gidx[:, :])
                hasb = cp.tile([128, NBLK], I32)
                nc.sync.dma_start(out=hasb[:], in_=halign[:, :])
                adsts = cp.tile([128, 2 * NBLK * H], FP16)
                for k in range(2 * NBLK):
                    nc.gpsimd.indirect_dma_start(
                        out=adsts[:, k * H:(k + 1) * H],
                        out_offset=None,
                        in_=adst[:, :],
                        in_offset=IndirectOffsetOnAxis(ap=gsb[:, k:k + 1], axis=0),
                    )
                knegs = cp.tile([128, 2 * NBLK * H], F32)
                nc.vector.tensor_tensor(
                    out=knegs[:].rearrange("p (b h) -> p b h", h=H),
                    in0=adsts[:].rearrange("p (b h) -> p b h", h=H),
                    in1=maxasb[:].rearrange("p (j h) -> p j h", j=1)
                        .to_broadcast([128, 2 * NBLK, H]),
                    op=ALU.add,
                )
                ksl = cp.tile([128, 2 * NBLK * H], F32)
                nc.vector.tensor_scalar_mul(ksl[:], knegs[:], SLOPE)
                nc.vector.tensor_tensor(out=knegs[:], in0=knegs[:], in1=ksl[:],
                                        op=ALU.max)
                nc.vector.tensor_scalar_mul(knegs[:], knegs[:], -1.0)

            if stages >= 3:
                # ---- phase 2: both passes ----
                phase2 = (tc.tile_pool(name="idxp", bufs=2),
                          tc.tile_pool(name="gath", bufs=2),
                          tc.tile_pool(name="attn", bufs=2),
                          tc.tile_pool(name="etile", bufs=2),
                          tc.tile_pool(name="mtile", bufs=2),
                          tc.tile_pool(name="ptile", bufs=3),
                          tc.tile_pool(name="ph3", bufs=2))
                ip, gp, apool, ep, mp, pp, p3 = (p.__enter__() for p in phase2)
                pools = dict(ip=ip, gp=gp, ap=apool, ep=ep, mp=mp, pp=pp)
                meta_sb = dict(idx=[idxL, idxH], kneg=knegs[:],
                               halign=hasb[:], adst=adsts[:])
                if stages >= 4:
                    emit_pass_blocks(nc, pools, (T_lo, T_hi), meta_sb, JH, 1, H,
                                     C, ROW1, PL, PHA, jcap=32)
                emit_pass_blocks(nc, pools, (T_lo, T_hi), meta_sb, JL, 0, H, C,
                                 ROW1, PL, PHA, jcap=32)

            if stages >= 5:
                # ---- phase 3: combine + elu + h2 ----
                b1sb = cp.tile([128, HC], F32)
                nc.sync.dma_start(out=b1sb[:], in_=b1r[:, :])
                w2sb = cp.tile([128, 2 * (NCLS + 2)], FP16)
                nc.sync.dma_start(out=w2sb[:, 0:NCLS + 2], in_=w2e[0:128, :])
                nc.sync.dma_start(out=w2sb[:, NCLS + 2:], in_=w2e[128:256, :])
                ident = cp.tile([128, 128], FP16)
                make_identity(nc, ident[:])
                for b in range(NBLK):
                    sl = p3.tile([128, RW], F32, tag="sl")
                    sh = p3.tile([128, RW], F32, tag="sh")
                    nc.sync.dma_start(out=sl[:], in_=PL[b * 128:(b + 1) * 128, :])
                    nc.sync.dma_start(out=sh[:], in_=PHA[b * 128:(b + 1) * 128, :])
                    nc.vector.tensor_tensor(out=sl[:], in0=sl[:], in1=sh[:],
                                            op=ALU.add)
                    nc.vector.tensor_scalar_add(sl[:, HC:HC + H],
                                                sl[:, HC:HC + H], 1e-12)
                    rec = p3.tile([128, H, 1], F32, tag="rec")
                    nc.vector.reciprocal(
                        rec[:, :, 0:1].rearrange("p h j -> p (h j)"),
                        sl[:, HC:HC + H])
                    nc.vector.tensor_scalar_mul(rec[:], rec[:], float(C))
                    o = p3.tile([128, HC], F32, tag="o")
                    nc.vector.tensor_tensor(
                        out=o[:].rearrange("p (h c) -> p h c", h=H),
                        in0=sl[:, 0:HC].rearrange("p (h c) -> p h c", h=H),
                        in1=rec[:].to_broadcast([128, H, C]),
                        op=ALU.mult,
                    )
                    nc.vector.tensor_tensor(out=o[:], in0=o[:], in1=b1sb[:],
                                            op=ALU.add)
                    # elu(o) = relu(o) + exp(min(o,0)) - 1
                    pos = p3.tile([128, HC], F32, tag="pos")
                    nc.scalar.activation(out=pos[:], in_=o[:], func=AF.Relu)
                    nc.vector.tensor_scalar_min(o[:], o[:], 0.0)
                    nc.scalar.activation(out=o[:], in_=o[:], func=AF.Exp)
                    nc.vector.tensor_tensor(out=o[:], in0=o[:], in1=pos[:],
                                            op=ALU.add)
                    elu = p3.tile([128, HC], FP16, tag="elu")
                    nc.vector.tensor_scalar_add(elu[:], o[:], -1.0)
                    ps2 = psp.tile([128, NCLS + 2], F32, tag="mm2")
                    for k in range(2):
                        pst = psp.tile([128, 128], FP16, tag="ptr")
                        nc.tensor.transpose(out=pst[:],
                                            in_=elu[:, k * 128:(k + 1) * 128],
                                            identity=ident[:])
                        eT = p3.tile([128, 128], FP16, tag="eT")
                        nc.vector.tensor_copy(out=eT[:], in_=pst[:])
                        nc.tensor.matmul(
                            out=ps2[:], lhsT=eT[:],
                            rhs=w2sb[:, k * (NCLS + 2):(k + 1) * (NCLS + 2)],
                            start=(k == 0), stop=(k == 1))
                    h2sb = p3.tile([128, NCLS + 2], F32, tag="h2sb")
                    nc.vector.tensor_copy(out=h2sb[:], in_=ps2[:])
                    nc.sync.dma_start(out=h2a[b * 128:(b + 1) * 128, :],
                                      in_=h2sb[:])
            else:
                zz = cp.tile([128, NCLS + 2], F32)
                nc.vector.memset(zz[:], 0.0)
                for b in range(NBLK):
                    nc.sync.dma_start(out=h2a[b * 128:(b + 1) * 128, :],
                                      in_=zz[:])
            if stages >= 3:
                for p in reversed(phase2):
                    p.__exit__(None, None, None)
    nc.compile()
    return nc


# --------------------------------------------------------------------------
# program 2: layer-2 aggregation + output
# --------------------------------------------------------------------------

def build_prog2(JL, JH, CL, CH):
    nc = bacc.Bacc("TRN2", target_bir_lowering=False, debug=False)
    t2lo = nc.declare_dram_parameter("T2_lo", [LO_ROWS, ROW2], FP16, isOutput=False)
    t2hi = nc.declare_dram_parameter("T2_hi", [HI_ROWS, ROW2], FP16, isOutput=False)
    idxL = nc.declare_dram_parameter("idxL", [128, CL], I16, isOutput=False)
    idxH = nc.declare_dram_parameter("idxH", [128, CH], I16, isOutput=False)
    kneg2 = nc.declare_dram_parameter("kneg2", [128, 2 * NBLK], F32, isOutput=False)
    adst2 = nc.declare_dram_parameter("adst2", [128, 2 * NBLK], F32, isOutput=False)
    halign = nc.declare_dram_parameter("halign", [128, NBLK], I32, isOutput=False)
    b2r = nc.declare_dram_parameter("b2rep", [128, NCLS], F32, isOutput=False)
    out2 = nc.declare_dram_parameter("out2", [NBLK * 128, NCLS], F32, isOutput=True)

    PL = nc.dram_tensor("PL2", [NBLK * 128, NCLS + 1], F32)
    PHA = nc.dram_tensor("PHA2", [NBLK * 128, NCLS + 1], F32)

    with TileContext(nc) as tc:
        with (
            tc.tile_pool(name="const", bufs=1) as cp,
            tc.tile_pool(name="idxp", bufs=2) as ip,
            tc.tile_pool(name="gath", bufs=3) as gp,
            tc.tile_pool(name="attn", bufs=2) as apool,
            tc.tile_pool(name="etile", bufs=2) as ep,
            tc.tile_pool(name="mtile", bufs=2) as mp,
            tc.tile_pool(name="ptile", bufs=3) as pp,
            tc.tile_pool(name="ph3", bufs=2) as p3,
        ):
            nc.gpsimd.load_library(library_config.mlp)
            knsb = cp.tile([128, 2 * NBLK], F32)
            nc.sync.dma_start(out=knsb[:], in_=kneg2[:, :])
            adsb = cp.tile([128, 2 * NBLK], F32)
            nc.sync.dma_start(out=adsb[:], in_=adst2[:, :])
            hasb = cp.tile([128, NBLK], I32)
            nc.sync.dma_start(out=hasb[:], in_=halign[:, :])
            pools = dict(ip=ip, gp=gp, ap=apool, ep=ep, mp=mp, pp=pp)
            meta_sb = dict(idx=[idxL, idxH], kneg=knsb[:],
                           halign=hasb[:], adst=adsb[:])
            emit_pass_blocks(nc, pools, (t2lo, t2hi), meta_sb, JH, 1, 1, NCLS, ROW2,
                             PL, PHA, jcap=64)
            emit_pass_blocks(nc, pools, (t2lo, t2hi), meta_sb, JL, 0, 1, NCLS, ROW2,
                             PL, PHA, jcap=64)

            b2sb = cp.tile([128, NCLS], F32)
            nc.sync.dma_start(out=b2sb[:], in_=b2r[:, :])
            for b in range(NBLK):
                sl = p3.tile([128, NCLS + 1], F32, tag="sl")
                sh = p3.tile([128, NCLS + 1], F32, tag="sh")
                nc.sync.dma_start(out=sl[:], in_=PL[b * 128:(b + 1) * 128, :])
                nc.sync.dma_start(out=sh[:], in_=PHA[b * 128:(b + 1) * 128, :])
                nc.vector.tensor_tensor(out=sl[:], in0=sl[:], in1=sh[:], op=ALU.add)
                nc.vector.tensor_scalar_add(sl[:, NCLS:NCLS + 1],
                                            sl[:, NCLS:NCLS + 1], 1e-12)
                rec = p3.tile([128, 1], F32, tag="rec")
                nc.vector.reciprocal(rec[:], sl[:, NCLS:NCLS + 1])
                nc.vector.tensor_scalar_mul(rec[:], rec[:], float(NCLS))
                o = p3.tile([128, NCLS], F32, tag="o")
                nc.vector.tensor_tensor(
                    out=o[:].rearrange("p (j c) -> p j c", j=1),
                    in0=sl[:, 0:NCLS].rearrange("p (j c) -> p j c", j=1),
                    in1=rec[:].rearrange("p (j c) -> p j c", j=1)
                        .to_broadcast([128, 1, NCLS]),
                    op=ALU.mult,
                )
                nc.vector.tensor_tensor(out=o[:], in0=o[:], in1=b2sb[:], op=ALU.add)
                nc.sync.dma_start(out=out2[b * 128:(b + 1) * 128, :], in_=o[:])
    nc.compile()
    return nc


# --------------------------------------------------------------------------
# host glue
# --------------------------------------------------------------------------

LAST_RESULTS = []


def kernel(x, edge_index, W1, att_src1, att_dst1, b1, W2, att_src2, att_dst2, b2,
           **_):
    LAST_RESULTS.clear()
    x = np.asarray(x, np.float32)
    edge_index = np.asarray(edge_index)
    plans = build_plans(edge_index)
    metas = host_meta(plans)
    JL = plans[0][0]["J"]
    JH = plans[0][1]["J"]
    CL = max(8 * int(JL.sum()), 16)
    CH = max(8 * int(JH.sum()), 16)

    W1 = np.asarray(W1, np.float32)
    W1a = np.einsum("fhc,hc->fh", W1.reshape(F_IN, H, C),
                    np.asarray(att_src1, np.float32))
    W1b = np.einsum("fhc,hc->fh", W1.reshape(F_IN, H, C),
                    np.asarray(att_dst1, np.float32))
    w1ext = np.concatenate([W1, W1a, W1b], axis=1).astype(np.float16)
    xT = np.zeros((F_IN, NPAD), np.float16)
    xT[:, :N] = x.T.astype(np.float16)
    a_src_host = x @ W1a
    maxasrc = np.tile((a_src_host.max(axis=0) + 0.05).astype(np.float32)[None, :],
                      (128, 1))

    W2 = np.asarray(W2, np.float32)
    W2a = W2 @ np.asarray(att_src2, np.float32).reshape(NCLS, 1)
    W2b = W2 @ np.asarray(att_dst2, np.float32).reshape(NCLS, 1)
    w2ext = np.concatenate([W2, W2a, W2b], axis=1).astype(np.float16)
    b1rep = np.tile(np.asarray(b1, np.float32)[None, :], (128, 1))
    b2rep = np.tile(np.asarray(b2, np.float32)[None, :], (128, 1))

    nc1 = build_prog1(JL, JH, CL, CH)
    in_maps = []
    for c in range(SH):
        m = metas[c]
        in_maps.append(dict(
            xT=xT, w1ext=w1ext, w2ext=w2ext, maxasrc=maxasrc, b1rep=b1rep,
            idxL=np.ascontiguousarray(m["idxL"]),
            idxH=np.ascontiguousarray(m["idxH"]),
            adst_gidx=np.ascontiguousarray(m["adst_gidx"]),
            halign=np.ascontiguousarray(m["h_align"]),
        ))
    res1 = run_bass_kernel_spmd(nc1, in_maps, core_ids=list(range(SH)))
    LAST_RESULTS.append(res1)

    # assemble full layer-2 node table on host
    h2_full = np.zeros((NPAD, NCLS + 2), np.float32)
    for c in range(SH):
        h2a = res1.results[c]["h2a"]
        order = plans[c][0]["order"].astype(np.int64)
        h2_full[order + c * NS] = h2a[:NS]
    rows2 = np.zeros((NPAD, ROW2), np.float16)
    rows2[:, :NCLS + 1] = h2_full[:, :NCLS + 1].astype(np.float16)
    T2_lo = np.zeros((LO_ROWS, ROW2), np.float16)
    T2_lo[1:] = rows2[:SPLIT]
    T2_lo[0, NCLS] = ASRC_DUMMY
    T2_hi = np.zeros((HI_ROWS, ROW2), np.float16)
    T2_hi[:HI_DUMMY] = rows2[SPLIT:]
    T2_hi[HI_DUMMY, NCLS] = ASRC_DUMMY
    a_dst2 = h2_full[:, NCLS + 1]
    max2 = float(h2_full[:N, NCLS].max()) + 0.05

    nc2 = build_prog2(JL, JH, CL, CH)
    in_maps2 = []
    for c in range(SH):
        m = metas[c]
        ad = a_dst2[np.concatenate([m["gidL"], m["gidH"]]).astype(np.int64)]
        ad = ad.reshape(2 * NBLK, 128).T.astype(np.float32)
        t = ad + max2
        kneg = -np.maximum(t, SLOPE * t)
        in_maps2.append(dict(
            T2_lo=T2_lo, T2_hi=T2_hi,
            idxL=np.ascontiguousarray(m["idxL"]),
            idxH=np.ascontiguousarray(m["idxH"]),
            kneg2=np.ascontiguousarray(kneg),
            adst2=np.ascontiguousarray(ad),
            halign=np.ascontiguousarray(m["h_align"]),
            b2rep=b2rep,
        ))
    res2 = run_bass_kernel_spmd(nc2, in_maps2, core_ids=list(range(SH)))
    LAST_RESULTS.append(res2)

    out = np.zeros((N, NCLS), np.float32)
    for c in range(SH):
        o2 = res2.results[c]["out2"]
        order = plans[c][0]["order"].astype(np.int64)
        out[order + c * NS] = o2[:NS]
    return out
